# revision 3
# baseline (speedup 1.0000x reference)
"""BPR pairwise softplus loss on 8 Trainium2 NeuronCores.

loss = mean_b sum_{i<K, j>=K, both valid} softplus(pred[b,j] - pred[b,i])

Strategy (data parallel over batch, 32 rows/core):
  softplus(n - p) = ln(1 + e^n * e^-p).
  - ScalarE: E = exp(pred_neg), F = exp(-pred_pos)  (small passes)
  - VectorE: zero out invalid entries (target == -1) so they contribute
    ln(1+0) = 0 exactly
  - TensorE: K=2 matmuls compute the outer product F_p * E_j for a pair of
    batch rows (128 partitions = 2x64 positives, free = 448 negatives) into
    PSUM. The stationary operand interleaves F with structural zeros so each
    partition half selects its own batch row.
  - ScalarE: big Ln(x + 1) passes over multiple PSUM banks with accum_out
    producing per-partition sums.
  - GpSimd: final partition reduction; host sums the 8 per-core scalars.
"""
import sys

sys.path.insert(0, "/opt/trn_rl_repo")

import numpy as np
import ml_dtypes

import concourse.bass as bass
import concourse.mybir as mybir
from concourse import bacc
from concourse.tile import TileContext
from concourse.bass_utils import run_bass_kernel_spmd

B, N, K = 256, 512, 64
NC = 8
RPC = B // NC            # 32 batch rows per core
NPAIR = RPC // 2         # 16 row-pair iterations
NEG = N - K              # 448 negatives per row
ACT_BATCHES = [1, 1, 2, 4, 4, 4]  # PSUM banks consumed per Ln pass

_PROG_CACHE = {}


def build_program(nreps: int = 1):
    """Build (and cache) the SPMD bass program. nreps>1 unrolls the body for
    throughput timing (independent replicas, outputs y[0, rep])."""
    if nreps in _PROG_CACHE:
        return _PROG_CACHE[nreps]
    nc = bacc.Bacc("TRN2", target_bir_lowering=False, debug=False, num_devices=NC)
    pred = nc.dram_tensor("pred", [RPC, N], mybir.dt.float32, kind="ExternalInput")
    tgt = nc.dram_tensor("tgt", [RPC, N], mybir.dt.int32, kind="ExternalInput")
    lz = nc.dram_tensor("lz", [2, 2 * K * NPAIR], mybir.dt.bfloat16,
                        kind="ExternalInput")  # zeros
    y = nc.dram_tensor("y", [1, nreps], mybir.dt.float32, kind="ExternalOutput")

    EXP = mybir.ActivationFunctionType.Exp
    LN = mybir.ActivationFunctionType.Ln

    with TileContext(nc) as tc:
        with (
            tc.tile_pool(name="io", bufs=2) as io,
            tc.tile_pool(name="mm", bufs=2) as mmp,
            tc.tile_pool(name="scr", bufs=2) as scr,
            tc.tile_pool(name="ps", bufs=2, space="PSUM") as ps,
        ):
            # Trigger the exp/ln activation-table load ASAP (~2.7us on
            # ScalarE, overlapping the input DMAs).
            d0 = io.tile([128, 1], mybir.dt.float32, tag="d0")
            nc.vector.memset(d0, 0.0)
            d1 = io.tile([128, 1], mybir.dt.bfloat16, tag="d1")
            nc.scalar.activation(d1, d0, EXP)

            for rep in range(nreps):
                pred_sb = io.tile([RPC, N], mybir.dt.float32, tag="pred")
                nc.sync.dma_start(out=pred_sb, in_=pred[:])
                tgt_sb = io.tile([RPC, N], mybir.dt.int32, tag="tgt")
                nc.sync.dma_start(out=tgt_sb, in_=tgt[:])

                # lhsT rows: 0 = F for rows 0..15 at q<64 (zeros at q>=64),
                #            1 = F for rows 16..31 at q>=64 (zeros at q<64)
                lhsT = mmp.tile([2, 2 * K * NPAIR], mybir.dt.bfloat16, tag="lhsT")
                nc.sync.dma_start(out=lhsT, in_=lz[:])
                # rhs rows: 0 = E row r, 1 = E row r+16
                rhs = mmp.tile([2, NEG * NPAIR], mybir.dt.bfloat16, tag="rhs")

                # E = exp(pred_neg), F = exp(-pred_pos)
                e_raw = io.tile([RPC, NEG], mybir.dt.bfloat16, tag="eraw")
                nc.scalar.activation(e_raw, pred_sb[:, K:N], EXP)
                f_raw = io.tile([RPC, K], mybir.dt.bfloat16, tag="fraw")
                nc.scalar.activation(f_raw, pred_sb[:, 0:K], EXP, scale=-1.0)

                # validity mask (0.0 where target == -1)
                valid = io.tile([RPC, N], mybir.dt.bfloat16, tag="valid")
                nc.vector.tensor_scalar(valid, tgt_sb, -1, None,
                                        mybir.AluOpType.not_equal)
                e_m = io.tile([RPC, NEG], mybir.dt.bfloat16, tag="em")
                nc.vector.tensor_mul(e_m, e_raw, valid[:, K:N])
                f_m = io.tile([RPC, K], mybir.dt.bfloat16, tag="fm")
                nc.vector.tensor_mul(f_m, f_raw, valid[:, 0:K])

                # de-interleave: rhs[t, r*448+j] = e_m[16*t + r, j]
                nc.sync.dma_start(
                    out=rhs.rearrange("t (r j) -> t r j", r=NPAIR),
                    in_=e_m)
                # pack F into the nonzero slots of lhsT
                nc.sync.dma_start(
                    out=lhsT[0:1, :].rearrange("p (r ti) -> p r ti",
                                               r=NPAIR)[:, :, 0:K],
                    in_=f_m[0:NPAIR, :])
                nc.sync.dma_start(
                    out=lhsT[1:2, :].rearrange("p (r ti) -> p r ti",
                                               r=NPAIR)[:, :, K:2 * K],
                    in_=f_m[NPAIR:RPC, :])

                npart = len(ACT_BATCHES)
                partials = mmp.tile([128, npart], mybir.dt.float32, tag="part")
                r = 0
                for bi, nb in enumerate(ACT_BATCHES):
                    pt = ps.tile([128, 4 * 512], mybir.dt.float32, tag="ps")
                    for k in range(nb):
                        # prod[p, j] = F[p-half row] * E[rowpair(p), j]
                        nc.tensor.matmul(
                            pt[:, 512 * k: 512 * k + NEG],
                            lhsT[:, 128 * r: 128 * (r + 1)],
                            rhs[:, NEG * r: NEG * (r + 1)],
                            start=True, stop=True)
                        r += 1
                    sout = scr.tile([128, 4 * NEG], mybir.dt.bfloat16, tag="scr")
                    nc.scalar.activation(
                        sout.rearrange("p (b x) -> p b x", x=NEG)[:, 0:nb, :],
                        pt.rearrange("p (b x) -> p b x", x=512)[:, 0:nb, 0:NEG],
                        LN, bias=1.0,
                        accum_out=partials[:, bi:bi + 1])

                tot = mmp.tile([1, 1], mybir.dt.float32, tag="tot")
                nc.gpsimd.tensor_reduce(tot, partials,
                                        mybir.AxisListType.XYZWC,
                                        mybir.AluOpType.add)
                nc.sync.dma_start(out=y[:, rep:rep + 1], in_=tot)

    nc.finalize()
    consts = _consts()
    _PROG_CACHE[nreps] = (nc, consts)
    return nc, consts


def _consts():
    lz = np.zeros((2, 2 * K * NPAIR), dtype=ml_dtypes.bfloat16)
    return (lz,)


def make_in_maps(prediction, target, consts):
    (lz,) = consts
    in_maps = []
    for c in range(NC):
        in_maps.append({
            "pred": np.ascontiguousarray(prediction[c * RPC:(c + 1) * RPC],
                                         dtype=np.float32),
            "tgt": np.ascontiguousarray(target[c * RPC:(c + 1) * RPC],
                                        dtype=np.int32),
            "lz": lz,
        })
    return in_maps


def kernel(prediction, target):
    nc, consts = build_program(1)
    in_maps = make_in_maps(prediction, target, consts)
    res = run_bass_kernel_spmd(nc, in_maps, core_ids=list(range(NC)))
    total = sum(float(res.results[c]["y"][0, 0]) for c in range(NC))
    return np.float32(total / B)


# revision 20
# speedup vs baseline: 1.3354x; 1.3354x over previous
"""BPR pairwise softplus loss on 8 Trainium2 NeuronCores.

loss = mean_b sum_{i<K, j>=K, both valid} softplus(pred[b,j] - pred[b,i])

Strategy (data parallel over batch, 32 rows/core):
  softplus(n - p) = ln(1 + e^n * e^-p).
  - ScalarE: E = exp(pred_neg), F = exp(-pred_pos)  (small passes)
  - VectorE: zero invalid entries (target == -1) so they contribute
    ln(1+0) = 0 exactly
  - TensorE: K=2 matmuls compute the outer product F_p * E_j for a pair of
    batch rows (128 partitions = 2x64 positives, free = 448 negatives) into
    PSUM; the stationary operand interleaves F with structural zeros so each
    partition half selects its own batch row (row r pairs with row r+16).
  - ScalarE: big Ln(x + 1) passes over up to 4 PSUM banks with accum_out
    producing per-partition sums.
  - GpSimd: final partition reduction; host sums the 8 per-core scalars.
"""
import sys

sys.path.insert(0, "/opt/trn_rl_repo")

import numpy as np
import ml_dtypes

import concourse.bass as bass
import concourse.mybir as mybir
from concourse import bacc
import concourse.hw_specs as hw_specs
from concourse.tile import TileContext
from concourse.bass_utils import run_bass_kernel_spmd

B, N, K = 256, 512, 64
NC = 8
RPC = B // NC            # 32 batch rows per core
NPAIR = RPC // 2         # 16 row-pair iterations
NEG = N - K              # 448 negatives per row
HNEG = NEG // 2          # 224 folded negative pairs
ACT_BATCHES = [2, 3, 3]  # PSUM banks consumed per Ln pass (2 pairs/bank)
N_WARM = 32  # PE warm-up dummy matmuls

_PROG_CACHE = {}

EXP = mybir.ActivationFunctionType.Exp
LN = mybir.ActivationFunctionType.Ln


def _patch_act_tables():
    """Make natural_log_exp_and_others the only table set advertising exp/ln
    so Bacc's table-load pass emits a single ACT_TABLE_LOAD instead of two
    (exp_and_others for the exps, then natural_log for the lns).  Set ids
    (dict order) are preserved; only advertised contents change."""
    if getattr(hw_specs.get_activation_tables, "_bpr_patched", False):
        return
    orig_fn = hw_specs.get_activation_tables

    def patched(arch):
        d = orig_fn(arch)
        out = {}
        for name, funcs in d.items():
            if name != "natural_log_exp_and_others" and (EXP in funcs
                                                         or LN in funcs):
                funcs = funcs - {EXP, LN}
            out[name] = funcs
        return out

    patched._bpr_patched = True
    hw_specs.get_activation_tables = patched
    bacc.get_activation_tables = patched


def build_program(nreps: int = 1):
    """Build (and cache) the SPMD bass program. nreps>1 unrolls the body for
    throughput timing (independent replicas, outputs y[0, rep])."""
    if nreps in _PROG_CACHE:
        return _PROG_CACHE[nreps]
    _patch_act_tables()
    nc = bacc.Bacc("TRN2", target_bir_lowering=False, debug=False, num_devices=NC)
    pred = nc.dram_tensor("pred", [RPC, N], mybir.dt.float32, kind="ExternalInput")
    tgt = nc.dram_tensor("tgt", [RPC, N], mybir.dt.int32, kind="ExternalInput")
    # 0/1 interleave pattern: rows 0..15 keep cols 0:64, rows 16..31 keep
    # cols 64:128 (compute engines need 32-aligned partition bases, so the
    # interleave is done by a full-height masked multiply instead of
    # half-height writes)
    pm = nc.dram_tensor("pm", [RPC, 2 * K], mybir.dt.bfloat16,
                        kind="ExternalInput")
    npart_g = len(ACT_BATCHES)
    y = nc.dram_tensor("y", [nreps, 128, npart_g], mybir.dt.float32,
                       kind="ExternalOutput")

    with TileContext(nc) as tc:
        with (
            tc.tile_pool(name="io", bufs=2) as io,
            tc.tile_pool(name="mm", bufs=2) as mmp,
            tc.tile_pool(name="scr", bufs=2) as scr,
            tc.tile_pool(name="ps", bufs=2, space="PSUM") as ps,
        ):
            # Trigger the exp/ln activation-table load ASAP (~2.7us on
            # ScalarE, overlapping the input DMAs).
            d0 = io.tile([128, 1], mybir.dt.float32, tag="d0")
            nc.vector.memset(d0, 0.0)
            d1 = io.tile([128, 1], mybir.dt.bfloat16, tag="d1")
            nc.scalar.activation(d1, d0, EXP)

            # Dummy matmuls keep TensorE continuously busy through the
            # prologue so it reaches full clock before the real matmuls.
            if N_WARM:
                dm = io.tile([2, 224], mybir.dt.bfloat16, tag="dm")
                nc.vector.memset(dm, 0.0)
                warm = ps.tile([128, 4 * 512], mybir.dt.float32, tag="ps")
                for _ in range(N_WARM):
                    nc.tensor.matmul(warm[:, 0:224], dm[:, 0:128], dm,
                                     start=True, stop=True)

            for rep in range(nreps):
                pred_sb = io.tile([RPC, N], mybir.dt.float32, tag="pred")
                nc.sync.dma_start(out=pred_sb, in_=pred[:])
                tgt_sb = io.tile([RPC, N], mybir.dt.int32, tag="tgt")
                nc.sync.dma_start(out=tgt_sb, in_=tgt[:])
                pm_sb = io.tile([RPC, 2 * K], mybir.dt.bfloat16, tag="pm")
                nc.sync.dma_start(out=pm_sb, in_=pm[:])

                # E = exp(pred_neg) first — the e-side chain is critical
                e_raw = io.tile([RPC, NEG], mybir.dt.bfloat16, tag="eraw")
                nc.scalar.activation(e_raw, pred_sb[:, K:N], EXP)

                # positives: shift invalid entries by +30 so exp(-x) ~ 0
                inv30 = io.tile([RPC, K], mybir.dt.float32, tag="inv30")
                nc.vector.tensor_scalar(inv30, tgt_sb[:, 0:K], -1, 30.0,
                                        mybir.AluOpType.is_equal,
                                        mybir.AluOpType.mult)
                parg = io.tile([RPC, K], mybir.dt.float32, tag="parg")
                nc.vector.tensor_add(parg, pred_sb[:, 0:K], inv30)

                # f4 row layout: [F interleaved (128) | F^2 interleaved (128)]
                # rows 0..15 hold F in cols 0:64, rows 16..31 in cols 64:128.
                f4 = io.tile([RPC, 4 * K], mybir.dt.bfloat16, tag="f4")
                nc.scalar.activation(f4[:, 0:K], parg, EXP, scale=-1.0)
                nc.scalar.activation(f4[:, K:2 * K], parg, EXP, scale=-1.0)
                nc.vector.tensor_mul(f4[:, 0:2 * K], f4[:, 0:2 * K], pm_sb)
                nc.vector.tensor_mul(f4[:, 2 * K:4 * K], f4[:, 0:2 * K],
                                     f4[:, 0:2 * K])

                # mask invalid entries (target == -1) to exact zero, fused:
                # e_m = (tgt != -1) * e_raw
                e_m = io.tile([RPC, NEG], mybir.dt.bfloat16, tag="em")
                nc.vector.scalar_tensor_tensor(
                    e_m, tgt_sb[:, K:N], -1, e_raw,
                    mybir.AluOpType.not_equal, mybir.AluOpType.mult)

                # fold adjacent negatives in pairs:
                # ln(1+x1) + ln(1+x2) = ln(1 + F*(E1+E2) + F^2*(E1*E2))
                # es_ep row = [E1+E2 (224) | E1*E2 (224)]
                es_ep = io.tile([RPC, NEG], mybir.dt.bfloat16, tag="esep")
                nc.vector.tensor_add(es_ep[:, 0:HNEG],
                                     e_m[:, 0:NEG:2], e_m[:, 1:NEG:2])
                nc.vector.tensor_mul(es_ep[:, HNEG:NEG],
                                     e_m[:, 0:NEG:2], e_m[:, 1:NEG:2])

                # deint: rhs[t, 448r + jj] = es_ep[16t + r, jj]
                rhs = mmp.tile([2, NEG * NPAIR], mybir.dt.bfloat16, tag="rhs")
                nc.sync.dma_start(
                    out=rhs.rearrange("t (r j) -> t r j", r=NPAIR),
                    in_=es_ep)
                # pack: lhsT_all[t, 256r + c] = f4[16t + r, c]
                lhsT_all = mmp.tile([2, 4 * K * NPAIR], mybir.dt.bfloat16,
                                    tag="lhsT")
                nc.sync.dma_start(
                    out=lhsT_all.rearrange("t (r c) -> t r c", r=NPAIR),
                    in_=f4)

                # 16 pairs at 224 floats -> 2 pairs per PSUM bank, 8 banks
                npart = len(ACT_BATCHES)
                partials = mmp.tile([128, npart], mybir.dt.float32, tag="part")
                r = 0
                for bi, nb in enumerate(ACT_BATCHES):
                    pt = ps.tile([128, 4 * 512], mybir.dt.float32, tag="ps")
                    for k in range(2 * nb):
                        # psum = F*esum, then += F^2*eprod  for rowpair(p)
                        out_sl = pt[:, 512 * (k // 2) + HNEG * (k % 2):
                                    512 * (k // 2) + HNEG * (k % 2) + HNEG]
                        nc.tensor.matmul(
                            out_sl,
                            lhsT_all[:, 256 * r: 256 * r + 128],
                            rhs[:, NEG * r: NEG * r + HNEG],
                            start=True, stop=False)
                        nc.tensor.matmul(
                            out_sl,
                            lhsT_all[:, 256 * r + 128: 256 * (r + 1)],
                            rhs[:, NEG * r + HNEG: NEG * (r + 1)],
                            start=False, stop=True)
                        r += 1
                    sout = scr.tile([128, 4 * 2 * HNEG], mybir.dt.bfloat16,
                                    tag="scr")
                    nc.scalar.activation(
                        sout.rearrange("p (b x) -> p b x",
                                       x=2 * HNEG)[:, 0:nb, :],
                        pt.rearrange("p (b x) -> p b x",
                                     x=512)[:, 0:nb, 0:2 * HNEG],
                        LN, bias=1.0,
                        accum_out=partials[:, bi:bi + 1])

                nc.sync.dma_start(out=y[rep], in_=partials)

    nc.finalize()
    _PROG_CACHE[nreps] = (nc, ())
    return nc, ()


def _pm_const():
    pmv = np.zeros((RPC, 2 * K), dtype=ml_dtypes.bfloat16)
    pmv[0:NPAIR, 0:K] = 1
    pmv[NPAIR:RPC, K:2 * K] = 1
    return pmv


def make_in_maps(prediction, target, consts):
    pmv = _pm_const()
    in_maps = []
    for c in range(NC):
        in_maps.append({
            "pred": np.ascontiguousarray(prediction[c * RPC:(c + 1) * RPC],
                                         dtype=np.float32),
            "tgt": np.ascontiguousarray(target[c * RPC:(c + 1) * RPC],
                                        dtype=np.int32),
            "pm": pmv,
        })
    return in_maps


def kernel(prediction, target):
    nc, consts = build_program(1)
    in_maps = make_in_maps(prediction, target, consts)
    res = run_bass_kernel_spmd(nc, in_maps, core_ids=list(range(NC)))
    total = sum(float(res.results[c]["y"][0].sum(dtype=np.float64))
                for c in range(NC))
    return np.float32(total / B)


# revision 22
# speedup vs baseline: 1.3589x; 1.0176x over previous
"""BPR pairwise softplus loss on 8 Trainium2 NeuronCores.

loss = mean_b sum_{i<K, j>=K, both valid} softplus(pred[b,j] - pred[b,i])

Strategy (data parallel over batch, 32 rows/core):
  softplus(n - p) = ln(1 + e^n * e^-p).
  - ScalarE: E = exp(pred_neg), F = exp(-pred_pos)  (small passes)
  - VectorE: zero invalid entries (target == -1) so they contribute
    ln(1+0) = 0 exactly
  - TensorE: K=2 matmuls compute the outer product F_p * E_j for a pair of
    batch rows (128 partitions = 2x64 positives, free = 448 negatives) into
    PSUM; the stationary operand interleaves F with structural zeros so each
    partition half selects its own batch row (row r pairs with row r+16).
  - ScalarE: big Ln(x + 1) passes over up to 4 PSUM banks with accum_out
    producing per-partition sums.
  - GpSimd: final partition reduction; host sums the 8 per-core scalars.
"""
import sys

sys.path.insert(0, "/opt/trn_rl_repo")

import numpy as np
import ml_dtypes

import concourse.bass as bass
import concourse.mybir as mybir
from concourse import bacc
import concourse.hw_specs as hw_specs
from concourse.tile import TileContext
from concourse.bass_utils import run_bass_kernel_spmd

B, N, K = 256, 512, 64
NC = 8
RPC = B // NC            # 32 batch rows per core
NPAIR = RPC // 2         # 16 row-pair iterations
NEG = N - K              # 448 negatives per row
HNEG = NEG // 2          # 224 folded negative pairs
ACT_BATCHES = [2, 3, 3]  # PSUM banks consumed per Ln pass (2 pairs/bank)
N_WARM = 32  # PE warm-up dummy matmuls

_PROG_CACHE = {}

EXP = mybir.ActivationFunctionType.Exp
LN = mybir.ActivationFunctionType.Ln


def _patch_act_tables():
    """Make natural_log_exp_and_others the only table set advertising exp/ln
    so Bacc's table-load pass emits a single ACT_TABLE_LOAD instead of two
    (exp_and_others for the exps, then natural_log for the lns).  Set ids
    (dict order) are preserved; only advertised contents change."""
    if getattr(hw_specs.get_activation_tables, "_bpr_patched", False):
        return
    orig_fn = hw_specs.get_activation_tables

    def patched(arch):
        d = orig_fn(arch)
        out = {}
        for name, funcs in d.items():
            if name != "natural_log_exp_and_others" and (EXP in funcs
                                                         or LN in funcs):
                funcs = funcs - {EXP, LN}
            out[name] = funcs
        return out

    patched._bpr_patched = True
    hw_specs.get_activation_tables = patched
    bacc.get_activation_tables = patched


def build_program(nreps: int = 1):
    """Build (and cache) the SPMD bass program. nreps>1 unrolls the body for
    throughput timing (independent replicas, outputs y[0, rep])."""
    if nreps in _PROG_CACHE:
        return _PROG_CACHE[nreps]
    _patch_act_tables()
    nc = bacc.Bacc("TRN2", target_bir_lowering=False, debug=False, num_devices=NC)
    pred = nc.dram_tensor("pred", [RPC, N], mybir.dt.float32, kind="ExternalInput")
    tgt = nc.dram_tensor("tgt", [RPC, N], mybir.dt.int32, kind="ExternalInput")
    # 0/1 interleave pattern: rows 0..15 keep cols 0:64, rows 16..31 keep
    # cols 64:128 (compute engines need 32-aligned partition bases, so the
    # interleave is done by a full-height masked multiply instead of
    # half-height writes)
    pm = nc.dram_tensor("pm", [RPC, 2 * K], mybir.dt.bfloat16,
                        kind="ExternalInput")
    npart_g = len(ACT_BATCHES)
    y = nc.dram_tensor("y", [nreps, 128, npart_g], mybir.dt.float32,
                       kind="ExternalOutput")

    with TileContext(nc) as tc:
        with (
            tc.tile_pool(name="io", bufs=2) as io,
            tc.tile_pool(name="mm", bufs=2) as mmp,
            tc.tile_pool(name="scr", bufs=2) as scr,
            tc.tile_pool(name="ps", bufs=2, space="PSUM") as ps,
        ):
            # Trigger the exp/ln activation-table load ASAP (~2.7us on
            # ScalarE, overlapping the input DMAs).
            d0 = io.tile([128, 1], mybir.dt.float32, tag="d0")
            nc.vector.memset(d0, 0.0)
            d1 = io.tile([128, 1], mybir.dt.bfloat16, tag="d1")
            nc.scalar.activation(d1, d0, EXP)

            # Dummy matmuls keep TensorE continuously busy through the
            # prologue so it reaches full clock before the real matmuls.
            if N_WARM:
                dm = io.tile([2, 224], mybir.dt.bfloat16, tag="dm")
                nc.vector.memset(dm, 0.0)
                warm = ps.tile([128, 4 * 512], mybir.dt.float32, tag="ps")
                for _ in range(N_WARM):
                    nc.tensor.matmul(warm[:, 0:224], dm[:, 0:128], dm,
                                     start=True, stop=True)

            for rep in range(nreps):
                pred_sb = io.tile([RPC, N], mybir.dt.float32, tag="pred")
                nc.sync.dma_start(out=pred_sb, in_=pred[:])
                tgt_sb = io.tile([RPC, N], mybir.dt.bfloat16, tag="tgt")
                nc.gpsimd.dma_start(out=tgt_sb, in_=tgt[:])
                pm_sb = io.tile([RPC, 2 * K], mybir.dt.bfloat16, tag="pm")
                nc.sync.dma_start(out=pm_sb, in_=pm[:])

                # E = exp(pred_neg) first — the e-side chain is critical
                e_raw = io.tile([RPC, NEG], mybir.dt.bfloat16, tag="eraw")
                nc.scalar.activation(e_raw, pred_sb[:, K:N], EXP)

                # positives: shift invalid entries by +30 so exp(-x) ~ 0
                inv30 = io.tile([RPC, K], mybir.dt.float32, tag="inv30")
                nc.vector.tensor_scalar(inv30, tgt_sb[:, 0:K], -1, 30.0,
                                        mybir.AluOpType.is_equal,
                                        mybir.AluOpType.mult)
                parg = io.tile([RPC, K], mybir.dt.float32, tag="parg")
                nc.vector.tensor_add(parg, pred_sb[:, 0:K], inv30)

                # f4 row layout: [F interleaved (128) | F^2 interleaved (128)]
                # rows 0..15 hold F in cols 0:64, rows 16..31 in cols 64:128.
                f4 = io.tile([RPC, 4 * K], mybir.dt.bfloat16, tag="f4")
                nc.scalar.activation(f4[:, 0:K], parg, EXP, scale=-1.0)
                nc.scalar.activation(f4[:, K:2 * K], parg, EXP, scale=-1.0)
                nc.vector.tensor_mul(f4[:, 0:2 * K], f4[:, 0:2 * K], pm_sb)
                nc.vector.tensor_mul(f4[:, 2 * K:4 * K], f4[:, 0:2 * K],
                                     f4[:, 0:2 * K])

                # mask invalid entries (target == -1) to exact zero, fused:
                # e_m = (tgt != -1) * e_raw
                e_m = io.tile([RPC, NEG], mybir.dt.bfloat16, tag="em")
                nc.vector.scalar_tensor_tensor(
                    e_m, tgt_sb[:, K:N], -1, e_raw,
                    mybir.AluOpType.not_equal, mybir.AluOpType.mult)

                # fold adjacent negatives in pairs:
                # ln(1+x1) + ln(1+x2) = ln(1 + F*(E1+E2) + F^2*(E1*E2))
                # es_ep row = [E1+E2 (224) | E1*E2 (224)]
                es_ep = io.tile([RPC, NEG], mybir.dt.bfloat16, tag="esep")
                nc.vector.tensor_add(es_ep[:, 0:HNEG],
                                     e_m[:, 0:HNEG], e_m[:, HNEG:NEG])
                nc.vector.tensor_mul(es_ep[:, HNEG:NEG],
                                     e_m[:, 0:HNEG], e_m[:, HNEG:NEG])

                # deint: rhs[t, 448r + jj] = es_ep[16t + r, jj]
                rhs = mmp.tile([2, NEG * NPAIR], mybir.dt.bfloat16, tag="rhs")
                nc.gpsimd.dma_start(
                    out=rhs.rearrange("t (r j) -> t r j", r=NPAIR),
                    in_=es_ep)
                # pack: lhsT_all[t, 256r + c] = f4[16t + r, c]
                lhsT_all = mmp.tile([2, 4 * K * NPAIR], mybir.dt.bfloat16,
                                    tag="lhsT")
                nc.sync.dma_start(
                    out=lhsT_all.rearrange("t (r c) -> t r c", r=NPAIR),
                    in_=f4)

                # 16 pairs at 224 floats -> 2 pairs per PSUM bank, 8 banks
                npart = len(ACT_BATCHES)
                partials = mmp.tile([128, npart], mybir.dt.float32, tag="part")
                r = 0
                for bi, nb in enumerate(ACT_BATCHES):
                    pt = ps.tile([128, 4 * 512], mybir.dt.float32, tag="ps")
                    for k in range(2 * nb):
                        # psum = F*esum, then += F^2*eprod  for rowpair(p)
                        out_sl = pt[:, 512 * (k // 2) + HNEG * (k % 2):
                                    512 * (k // 2) + HNEG * (k % 2) + HNEG]
                        nc.tensor.matmul(
                            out_sl,
                            lhsT_all[:, 256 * r: 256 * r + 128],
                            rhs[:, NEG * r: NEG * r + HNEG],
                            start=True, stop=False)
                        nc.tensor.matmul(
                            out_sl,
                            lhsT_all[:, 256 * r + 128: 256 * (r + 1)],
                            rhs[:, NEG * r + HNEG: NEG * (r + 1)],
                            start=False, stop=True)
                        r += 1
                    sout = scr.tile([128, 4 * 2 * HNEG], mybir.dt.bfloat16,
                                    tag="scr")
                    nc.scalar.activation(
                        sout.rearrange("p (b x) -> p b x",
                                       x=2 * HNEG)[:, 0:nb, :],
                        pt.rearrange("p (b x) -> p b x",
                                     x=512)[:, 0:nb, 0:2 * HNEG],
                        LN, bias=1.0,
                        accum_out=partials[:, bi:bi + 1])

                nc.sync.dma_start(out=y[rep], in_=partials)

    nc.finalize()
    _PROG_CACHE[nreps] = (nc, ())
    return nc, ()


def _pm_const():
    pmv = np.zeros((RPC, 2 * K), dtype=ml_dtypes.bfloat16)
    pmv[0:NPAIR, 0:K] = 1
    pmv[NPAIR:RPC, K:2 * K] = 1
    return pmv


def make_in_maps(prediction, target, consts):
    pmv = _pm_const()
    in_maps = []
    for c in range(NC):
        in_maps.append({
            "pred": np.ascontiguousarray(prediction[c * RPC:(c + 1) * RPC],
                                         dtype=np.float32),
            "tgt": np.ascontiguousarray(target[c * RPC:(c + 1) * RPC],
                                        dtype=np.int32),
            "pm": pmv,
        })
    return in_maps


def kernel(prediction, target):
    nc, consts = build_program(1)
    in_maps = make_in_maps(prediction, target, consts)
    res = run_bass_kernel_spmd(nc, in_maps, core_ids=list(range(NC)))
    total = sum(float(res.results[c]["y"][0].sum(dtype=np.float64))
                for c in range(NC))
    return np.float32(total / B)


# revision 24
# speedup vs baseline: 1.3786x; 1.0144x over previous
"""BPR pairwise softplus loss on 8 Trainium2 NeuronCores.

loss = mean_b sum_{i<K, j>=K, both valid} softplus(pred[b,j] - pred[b,i])

Strategy (data parallel over batch, 32 rows/core), using
  softplus(n - p) = ln(1 + e^n * e^-p)
and folding two negatives per ln via
  ln(1+x1) + ln(1+x2) = ln(1 + F*(E1+E2) + F^2*(E1*E2)),  x_k = F*E_k:

  - ScalarE: E = exp(pred_neg) [zeroed where invalid], F = exp(-pred_pos)
    [invalids pushed to ~0 via a +30 shift], then big Ln(x + 1) passes over
    multiple PSUM banks with accum_out producing per-partition row sums.
    One activation-table load (exp+ln share natural_log_exp_and_others).
  - VectorE: masking, E pair sums/products, F interleave / squares.
  - TensorE: per row-pair r (row r with row r+16; 128 partitions = 2x64
    positives) two accumulating K=2 matmuls build
    psum = F*(E1+E2) + F^2*(E1*E2) (free dim = 224 folded negative pairs);
    the stationary operand interleaves F with structural zeros so each
    partition half selects its own batch row.  Dummy matmuls warm the PE
    clock during the prologue.
  - per-partition partial sums are DMA'd out; the host sums 8x128x3 partials
    and divides by B (the unshard/all-reduce step).
"""
import sys

sys.path.insert(0, "/opt/trn_rl_repo")

import numpy as np
import ml_dtypes

import concourse.bass as bass
import concourse.mybir as mybir
from concourse import bacc
import concourse.hw_specs as hw_specs
from concourse.tile import TileContext
from concourse.bass_utils import run_bass_kernel_spmd

B, N, K = 256, 512, 64
NC = 8
RPC = B // NC            # 32 batch rows per core
NPAIR = RPC // 2         # 16 row-pair iterations
NEG = N - K              # 448 negatives per row
HNEG = NEG // 2          # 224 folded negative pairs
ACT_BATCHES = [2, 3, 3]  # PSUM banks consumed per Ln pass (2 pairs/bank)
N_WARM = 32  # PE warm-up dummy matmuls

_PROG_CACHE = {}

EXP = mybir.ActivationFunctionType.Exp
LN = mybir.ActivationFunctionType.Ln


def _patch_act_tables():
    """Make natural_log_exp_and_others the only table set advertising exp/ln
    so Bacc's table-load pass emits a single ACT_TABLE_LOAD instead of two
    (exp_and_others for the exps, then natural_log for the lns).  Set ids
    (dict order) are preserved; only advertised contents change."""
    if getattr(hw_specs.get_activation_tables, "_bpr_patched", False):
        return
    orig_fn = hw_specs.get_activation_tables

    def patched(arch):
        d = orig_fn(arch)
        out = {}
        for name, funcs in d.items():
            if name != "natural_log_exp_and_others" and (EXP in funcs
                                                         or LN in funcs):
                funcs = funcs - {EXP, LN}
            out[name] = funcs
        return out

    patched._bpr_patched = True
    hw_specs.get_activation_tables = patched
    bacc.get_activation_tables = patched


def build_program(nreps: int = 1):
    """Build (and cache) the SPMD bass program. nreps>1 unrolls the body for
    throughput timing (independent replicas, outputs y[0, rep])."""
    if nreps in _PROG_CACHE:
        return _PROG_CACHE[nreps]
    _patch_act_tables()
    nc = bacc.Bacc("TRN2", target_bir_lowering=False, debug=False, num_devices=NC)
    pred = nc.dram_tensor("pred", [RPC, N], mybir.dt.float32, kind="ExternalInput")
    tgt = nc.dram_tensor("tgt", [RPC, N], mybir.dt.int32, kind="ExternalInput")
    # 0/1 interleave pattern: rows 0..15 keep cols 0:64, rows 16..31 keep
    # cols 64:128 (compute engines need 32-aligned partition bases, so the
    # interleave is done by a full-height masked multiply instead of
    # half-height writes)
    pm = nc.dram_tensor("pm", [RPC, 2 * K], mybir.dt.bfloat16,
                        kind="ExternalInput")
    npart_g = len(ACT_BATCHES)
    y = nc.dram_tensor("y", [nreps, 128, npart_g], mybir.dt.float32,
                       kind="ExternalOutput")

    with TileContext(nc) as tc:
        with (
            tc.tile_pool(name="io", bufs=2) as io,
            tc.tile_pool(name="mm", bufs=2) as mmp,
            tc.tile_pool(name="scr", bufs=2) as scr,
            tc.tile_pool(name="ps", bufs=2, space="PSUM") as ps,
        ):
            # Trigger the exp/ln activation-table load ASAP (~2.7us on
            # ScalarE, overlapping the input DMAs).
            d0 = io.tile([128, 1], mybir.dt.float32, tag="d0")
            nc.vector.memset(d0, 0.0)
            d1 = io.tile([128, 1], mybir.dt.bfloat16, tag="d1")
            nc.scalar.activation(d1, d0, EXP)

            # Dummy matmuls keep TensorE continuously busy through the
            # prologue so it reaches full clock before the real matmuls.
            if N_WARM:
                dm = io.tile([2, 224], mybir.dt.bfloat16, tag="dm")
                nc.vector.memset(dm, 0.0)
                warm = ps.tile([128, 4 * 512], mybir.dt.float32, tag="ps")
                for _ in range(N_WARM):
                    nc.tensor.matmul(warm[:, 0:224], dm[:, 0:128], dm,
                                     start=True, stop=True)

            for rep in range(nreps):
                pred_sb = io.tile([RPC, N], mybir.dt.float32, tag="pred")
                nc.sync.dma_start(out=pred_sb, in_=pred[:])
                tgt_sb = io.tile([RPC, N], mybir.dt.bfloat16, tag="tgt")
                nc.gpsimd.dma_start(out=tgt_sb, in_=tgt[:])
                pm_sb = io.tile([RPC, 2 * K], mybir.dt.bfloat16, tag="pm")
                nc.sync.dma_start(out=pm_sb, in_=pm[:])

                # E = exp(pred_neg) first — the e-side chain is critical
                e_raw = io.tile([RPC, NEG], mybir.dt.bfloat16, tag="eraw")
                nc.scalar.activation(e_raw, pred_sb[:, K:N], EXP)

                # positives: shift invalid entries by +30 so exp(-x) ~ 0
                inv30 = io.tile([RPC, K], mybir.dt.float32, tag="inv30")
                nc.vector.tensor_scalar(inv30, tgt_sb[:, 0:K], -1, 30.0,
                                        mybir.AluOpType.is_equal,
                                        mybir.AluOpType.mult)
                parg = io.tile([RPC, K], mybir.dt.float32, tag="parg")
                nc.vector.tensor_add(parg, pred_sb[:, 0:K], inv30)

                # f4 row layout: [F interleaved (128) | F^2 interleaved (128)]
                # rows 0..15 hold F in cols 0:64, rows 16..31 in cols 64:128.
                f4 = io.tile([RPC, 4 * K], mybir.dt.bfloat16, tag="f4")
                nc.scalar.activation(f4[:, 0:K], parg, EXP, scale=-1.0)
                nc.scalar.activation(f4[:, K:2 * K], parg, EXP, scale=-1.0)
                nc.vector.tensor_mul(f4[:, 0:2 * K], f4[:, 0:2 * K], pm_sb)
                nc.vector.tensor_mul(f4[:, 2 * K:4 * K], f4[:, 0:2 * K],
                                     f4[:, 0:2 * K])

                # mask invalid entries (target == -1) to exact zero;
                # valid precomputes off the exp critical path
                vneg = io.tile([RPC, NEG], mybir.dt.bfloat16, tag="vneg")
                nc.vector.tensor_scalar(vneg, tgt_sb[:, K:N], -1, None,
                                        mybir.AluOpType.not_equal)
                e_m = io.tile([RPC, NEG], mybir.dt.bfloat16, tag="em")
                nc.vector.tensor_mul(e_m, e_raw, vneg)

                # fold adjacent negatives in pairs:
                # ln(1+x1) + ln(1+x2) = ln(1 + F*(E1+E2) + F^2*(E1*E2))
                # es_ep row = [E1+E2 (224) | E1*E2 (224)]
                es_ep = io.tile([RPC, NEG], mybir.dt.bfloat16, tag="esep")
                nc.vector.tensor_add(es_ep[:, 0:HNEG],
                                     e_m[:, 0:HNEG], e_m[:, HNEG:NEG])
                nc.vector.tensor_mul(es_ep[:, HNEG:NEG],
                                     e_m[:, 0:HNEG], e_m[:, HNEG:NEG])

                # deint: rhs[t, 448r + jj] = es_ep[16t + r, jj]
                rhs = mmp.tile([2, NEG * NPAIR], mybir.dt.bfloat16, tag="rhs")
                nc.gpsimd.dma_start(
                    out=rhs.rearrange("t (r j) -> t r j", r=NPAIR),
                    in_=es_ep)
                # pack: lhsT_all[t, 256r + c] = f4[16t + r, c]
                lhsT_all = mmp.tile([2, 4 * K * NPAIR], mybir.dt.bfloat16,
                                    tag="lhsT")
                nc.sync.dma_start(
                    out=lhsT_all.rearrange("t (r c) -> t r c", r=NPAIR),
                    in_=f4)

                # 16 pairs at 224 floats -> 2 pairs per PSUM bank, 8 banks
                npart = len(ACT_BATCHES)
                partials = mmp.tile([128, npart], mybir.dt.float32, tag="part")
                r = 0
                for bi, nb in enumerate(ACT_BATCHES):
                    pt = ps.tile([128, 4 * 512], mybir.dt.float32, tag="ps")
                    for k in range(2 * nb):
                        # psum = F*esum, then += F^2*eprod  for rowpair(p)
                        out_sl = pt[:, 512 * (k // 2) + HNEG * (k % 2):
                                    512 * (k // 2) + HNEG * (k % 2) + HNEG]
                        nc.tensor.matmul(
                            out_sl,
                            lhsT_all[:, 256 * r: 256 * r + 128],
                            rhs[:, NEG * r: NEG * r + HNEG],
                            start=True, stop=False)
                        nc.tensor.matmul(
                            out_sl,
                            lhsT_all[:, 256 * r + 128: 256 * (r + 1)],
                            rhs[:, NEG * r + HNEG: NEG * (r + 1)],
                            start=False, stop=True)
                        r += 1
                    sout = scr.tile([128, 4 * 2 * HNEG], mybir.dt.bfloat16,
                                    tag="scr")
                    nc.scalar.activation(
                        sout.rearrange("p (b x) -> p b x",
                                       x=2 * HNEG)[:, 0:nb, :],
                        pt.rearrange("p (b x) -> p b x",
                                     x=512)[:, 0:nb, 0:2 * HNEG],
                        LN, bias=1.0,
                        accum_out=partials[:, bi:bi + 1])

                nc.sync.dma_start(out=y[rep], in_=partials)

    nc.finalize()
    _PROG_CACHE[nreps] = (nc, ())
    return nc, ()


def _pm_const():
    pmv = np.zeros((RPC, 2 * K), dtype=ml_dtypes.bfloat16)
    pmv[0:NPAIR, 0:K] = 1
    pmv[NPAIR:RPC, K:2 * K] = 1
    return pmv


def make_in_maps(prediction, target, consts):
    pmv = _pm_const()
    in_maps = []
    for c in range(NC):
        in_maps.append({
            "pred": np.ascontiguousarray(prediction[c * RPC:(c + 1) * RPC],
                                         dtype=np.float32),
            "tgt": np.ascontiguousarray(target[c * RPC:(c + 1) * RPC],
                                        dtype=np.int32),
            "pm": pmv,
        })
    return in_maps


def kernel(prediction, target):
    nc, consts = build_program(1)
    in_maps = make_in_maps(prediction, target, consts)
    res = run_bass_kernel_spmd(nc, in_maps, core_ids=list(range(NC)))
    total = sum(float(res.results[c]["y"][0].sum(dtype=np.float64))
                for c in range(NC))
    return np.float32(total / B)
